# revision 1
# baseline (speedup 1.0000x reference)
"""EuclideanDeconf kernel for 8x TRN2 NeuronCores.

Computes out[b, c] = (2/D) * x @ W.T - ||x||^2/D - ||W||^2/D
for x [16384, 1024] f32, W [2048, 1024] f32 -> out [16384, 2048] f32.

Sharding: data-parallel over the batch dim. Each of the 8 cores gets 2048
rows of x (passed pre-transposed as xT [1024, 2048] f32) and the full W
(passed pre-transposed, scaled by 16 and e4m3-cast as wT [1024, 2048]).
The host does layout-only work (transpose / cast / shard / concat); all
FLOPs (matmul, row/col norms, combine) run on device.

Numerics (default fp8 mode): the cross term's magnitude is only ~0.003 of
the ~1.0 output (which is dominated by -||x||^2/D), so e4m3 rounding of the
matmul operands contributes only ~1e-4 relative error to the output. x2 is
computed on-device in fp32 from the fp32 x (the dominant term, kept exact);
w2 from e4m3 W (w2 is ~0.002, so its rounding is ~1e-5 absolute). Measured
vs the fp32 reference: max rel err 6.2e-4, norm rel err 1.0e-4. The bf16
mode (K_MM=bf16) gives max rel err 4e-5 at ~20% more runtime.

Engine assignment (per core, fp8 mode, HW ~122us):
  PE:     256 e4m3 DoubleRow matmuls (K=256 per op; the 8.6 GFLOP core of
          the op) + 32 w2-reduce + 4 w2-replicate + 16 tiny x2-dot matmuls
          + warmup (dummy matmuls so the PE HAM clock-gate releases early)
  ACT:    W^2 squares, psum->sbuf copy-outs, epilogue pass 1:
          t = (2/(16D))*psum - x2[b]  (scale + per-partition bias)
  DVE:    x f32->fp8 casts, x2 k-add-trees, epilogue pass 2: y = t - w2[c]
  GPSIMD: x^2 squares only
  DMA:    everything on the SP (sync) HWDGE ring; x chunk 0 first, W second

All engines execute their queues strictly in program order, so emission
order is the schedule: the w2 chain (W DMA -> wsq -> reduce -> replicate)
is emitted before chunk 0's b-tile groups (its DVE copies must not queue
behind epilogue adds that depend on them), and x2 columns are produced
per-b-tile so ACT can drain PSUM as soon as each accumulation closes.
Variants measured and rejected on HW: rank-1 w2-fold into PSUM (+13.6us
PE, made PE the bottleneck again: 136us), SWDGE cast-DMA for x8 (SWDGE
cast path is ~5x too slow: 153us), x2 trees on gpsimd (per-op overhead:
152us), chunk-0 matmuls emitted before the w2 chain (130us).
"""

import numpy as np
import ml_dtypes

# Problem constants (hardcoded; kernel.py must be self-contained).
B, D, C = 16384, 1024, 2048
NCORES = 8
BSH = B // NCORES  # 2048 rows of x per core
P = 128            # partitions
KT = D // P        # 8 contraction tiles
BCH = 512          # b-chunk (columns of xT loaded per DMA)

_CACHE = {}

import os as _os

# "bf16": plain bf16 matmuls (max rel err ~4e-5, HW ~164us)
# "fp8": e4m3 + DoubleRow matmuls (max rel err ~6e-4, HW ~122us)
MM_MODE = _os.environ.get("K_MM", "fp8")


def _build_nc():
    import concourse.tile as tile
    import concourse.mybir as mybir
    import concourse.bass as bass
    from concourse import bacc

    f32 = mybir.dt.float32
    bf16 = mybir.dt.bfloat16
    PSUM = bass.MemorySpace.PSUM
    Identity = mybir.ActivationFunctionType.Identity
    Copy = mybir.ActivationFunctionType.Copy
    MULT = mybir.AluOpType.mult
    ADD = mybir.AluOpType.add

    fp8 = MM_MODE == "fp8"
    mdt = mybir.dt.float8e4 if fp8 else bf16   # matmul operand dtype
    # In fp8 mode W is host-prescaled by 16 (keeps values out of the e4m3
    # subnormal range); the epilogue scale folds the 1/16 back out.
    cross_scale = 2.0 / D / (16.0 if fp8 else 1.0)
    w2_scale = 1.0 / D / (256.0 if fp8 else 1.0)
    DR = mybir.MatmulPerfMode.DoubleRow if fp8 else None

    nc = bacc.Bacc(
        "TRN2",
        target_bir_lowering=False,
        debug=False,
        enable_asserts=False,
    )
    xT = nc.dram_tensor("xT", [D, BSH], f32, kind="ExternalInput").ap()
    wT = nc.dram_tensor("wT", [D, C], mdt, kind="ExternalInput").ap()
    y = nc.dram_tensor("y", [BSH, C], f32, kind="ExternalOutput").ap()

    with tile.TileContext(nc) as tc:
        with (
            tc.tile_pool(name="consts", bufs=1) as cpool,
            tc.tile_pool(name="wpool", bufs=1) as wpool,
            tc.tile_pool(name="xpool", bufs=2) as xpool,
            tc.tile_pool(name="xsqpool", bufs=3) as xsqpool,
            tc.tile_pool(name="epool", bufs=8) as epool,
            tc.tile_pool(name="ypool", bufs=3) as ypool,
            tc.tile_pool(name="spool", bufs=8) as spool,
            tc.tile_pool(name="pmain", bufs=3, space=PSUM) as pmain,
            tc.tile_pool(name="psmall", bufs=1, space=PSUM) as psmall,
        ):
            negones_f = cpool.tile([P, 1], f32)
            nc.gpsimd.memset(negones_f[:], -1.0)
            negones_b = cpool.tile([P, 1], bf16)
            nc.gpsimd.memset(negones_b[:], -1.0)
            ones1_b = cpool.tile([1, P], bf16)
            nc.gpsimd.memset(ones1_b[:], 1.0)
            warm = cpool.tile([1, 1], f32)
            # touch ACT early so its function-table DMA (~2.7us) is off the
            # critical path by the time the first epilogue runs
            nc.scalar.activation(warm[:], negones_f[0:1, 0:1], Identity,
                                 bias=0.0, scale=1.0)

            # ---- PE warmup: dummy matmuls so HAM un-throttles (and the PE
            # is at 2.4 GHz) by the time real work arrives ----
            warm_b = cpool.tile([P, 512], bf16)
            nc.gpsimd.memset(warm_b[:], 0.0)
            warm_ps = psmall.tile([P, 512], f32, tag="w2ps", bufs=1)
            for _ in range(20):
                nc.tensor.matmul(warm_ps[:], warm_b[:, 0:P], warm_b[:],
                                 start=True, stop=True)

            wbf = wpool.tile([P, KT, C], mdt)
            wTr = wT.rearrange("(k p) c -> p k c", p=P)

            y_bufs = {}

            def btile_matmuls(jg, xbf, jl):
                """Issue the 32 accumulating matmuls for one 128-row b-tile."""
                y_t = ypool.tile([P, C], f32, tag="y_t", name=f"y_t{jg}")
                ps0 = pmain.tile([P, 1024], f32, tag="ps", name=f"ps{jg}a")
                ps1 = pmain.tile([P, 1024], f32, tag="ps", name=f"ps{jg}b")
                pss = (ps0, ps0, ps1, ps1)
                if fp8:
                    for k2 in range(KT // 2):
                        lhsT = xbf[:, 2 * k2:2 * k2 + 2, jl * P:(jl + 1) * P]
                        for cj in range(4):
                            nc.tensor.matmul(
                                pss[cj][:, (cj % 2) * 512:(cj % 2) * 512 + 512],
                                lhsT,
                                wbf[:, 2 * k2:2 * k2 + 2, cj * 512:(cj + 1) * 512],
                                start=(k2 == 0),
                                stop=(k2 == KT // 2 - 1),
                                perf_mode=DR,
                            )
                else:
                    for k in range(KT):
                        lhsT = xbf[:, k, jl * P:(jl + 1) * P]
                        for cj in range(4):
                            nc.tensor.matmul(
                                pss[cj][:, (cj % 2) * 512:(cj % 2) * 512 + 512],
                                lhsT,
                                wbf[:, k, cj * 512:(cj + 1) * 512],
                                start=(k == 0),
                                stop=(k == KT - 1),
                            )
                y_bufs[jg] = (y_t, ps0, ps1)

            def x2_tree(xsq, tag):
                """k-add-tree for one b-tile's x^2 partials (DVE)."""
                t4 = xsqpool.tile([P, 4, P], f32, tag="t4", name=f"t4_{tag}")
                nc.vector.tensor_tensor(t4[:], xsq[:, 0:4, :], xsq[:, 4:8, :],
                                        op=ADD)
                t2 = xsqpool.tile([P, 2, P], f32, tag="t2", name=f"t2_{tag}")
                nc.vector.tensor_tensor(t2[:], t4[:, 0:2, :], t4[:, 2:4, :],
                                        op=ADD)
                t1 = xsqpool.tile([P, P], f32, tag="t1", bufs=8,
                                  name=f"t1_{tag}")
                nc.vector.tensor_tensor(t1[:], t2[:, 0, :], t2[:, 1, :], op=ADD)
                return t1

            def x2_col(t1, tag):
                """x2 column (-sum(x^2)/D) for one b-tile: PE dot + ACT copy."""
                x2ps = psmall.tile([P, 1], f32, tag="x2ps", bufs=1,
                                   name=f"x2ps{tag}")
                nc.tensor.matmul(x2ps[:], t1[:], negones_f[:],
                                 start=True, stop=True)
                x2c = spool.tile([P, 1], f32, tag="x2c", name=f"x2c{tag}")
                # copy-out on ACT (idle early; DVE is busy with casts/wsq)
                nc.scalar.activation(x2c[:], x2ps[:], Copy, bias=0.0,
                                     scale=1.0 / D)
                return x2c

            def prep(ch):
                """DMA + cast + x^2 squares + k-trees for one chunk (all the
                per-chunk work with no PSUM/epilogue dependencies), emitted
                ahead of compute so the in-order DVE/GPSIMD queues never make
                the PE wait at a chunk boundary."""
                xTr = xT[:, ch * BCH:(ch + 1) * BCH].rearrange(
                    "(k p) b -> p k b", p=P
                )
                xf = xpool.tile([P, KT, BCH], f32, tag="xf", name=f"xf{ch}")
                xbf = xpool.tile([P, KT, BCH], mdt, tag="xbf",
                                 bufs=(3 if fp8 else 2), name=f"xbf{ch}")
                if ch == 0:
                    for k in range(KT):
                        nc.sync.dma_start(xf[:, k, :], xTr[:, k, :])
                        nc.vector.tensor_copy(xbf[:, k, :], xf[:, k, :])
                else:
                    nc.sync.dma_start(xf[:], xTr)
                    nc.vector.tensor_copy(xbf[:], xf[:])
                t1s = []
                for jj in range(BCH // P):
                    sl = slice(jj * P, (jj + 1) * P)
                    xsq = xsqpool.tile([P, KT, P], f32, tag="xsq",
                                       name=f"xsq{ch}_{jj}")
                    nc.gpsimd.tensor_tensor(xsq[:], xf[:, :, sl],
                                            xf[:, :, sl], op=MULT)
                    t1s.append(x2_tree(xsq, f"c{ch}_{jj}"))
                return xbf, t1s

            def compute(ch, xbf, t1s):
                for jj in range(BCH // P):
                    j = ch * (BCH // P) + jj
                    btile_matmuls(j, xbf, jj)
                    x2c = x2_col(t1s[jj], f"c{ch}_{jj}")
                    btile_epilogue(j, x2c, w2rep, split=(j == BSH // P - 1))

            def btile_epilogue(jg, x2c, w2rep, split=False):
                y_t, ps0, ps1 = y_bufs.pop(jg)
                for h, psh in enumerate((ps0, ps1)):
                    ysl = y_t[:, h * 1024:(h + 1) * 1024]
                    t = epool.tile([P, 1024], f32, tag="t", name=f"t{jg}_{h}")
                    # t = cross_scale*psum - x2  (scale + per-partition bias)
                    nc.scalar.activation(t[:], psh[:], Identity,
                                         bias=x2c[:], scale=cross_scale)
                    # y = t - w2  (w2rep already negated)
                    nc.vector.tensor_add(
                        ysl, t[:], w2rep[:, h * 1024:(h + 1) * 1024]
                    )
                    if split:
                        # last b-tile: store each half as soon as it's ready
                        # so the final DMA overlaps the second half's epilogue
                        nc.sync.dma_start(
                            y[jg * P:(jg + 1) * P, h * 1024:(h + 1) * 1024],
                            ysl,
                        )
                if not split:
                    nc.sync.dma_start(y[jg * P:(jg + 1) * P, :], y_t[:])

            # ---- chunk 0 prep first (x pieces lead on the sync ring so
            # casts/squares start immediately), then W ----
            xbf0, t1s0 = prep(0)
            for k in range(KT):
                nc.sync.dma_start(wbf[:, k, :], wTr[:, k, :])

            # ---- w2: squares, partition reduce on PE ----
            # fp8: squares on ACT (DVE is the scarce engine); w2row becomes a
            #      bf16 row folded into each b-tile's PSUM via rank-1 matmuls.
            # bf16: squares on DVE; w2row replicated to [128, C] f32 for the
            #      DVE epilogue-subtract pass.
            wsq = wpool.tile([P, KT, C], bf16)
            Square = mybir.ActivationFunctionType.Square
            for k in range(KT):
                if fp8:
                    nc.scalar.activation(wsq[:, k, :], wbf[:, k, :], Square)
                else:
                    nc.vector.tensor_tensor(wsq[:, k, :], wbf[:, k, :],
                                            wbf[:, k, :], op=MULT)
            w2row = wpool.tile([1, C], bf16)
            for cj in range(C // 512):
                w2ps = psmall.tile([1, 512], f32, tag="w2ps", bufs=1,
                                   name=f"w2ps{cj}")
                for k in range(KT):
                    nc.tensor.matmul(
                        w2ps[:],
                        negones_b[:],
                        wsq[:, k, cj * 512:(cj + 1) * 512],
                        start=(k == 0),
                        stop=(k == KT - 1),
                    )
                # w2row = -sum(W^2)/D (bf16 row; its values are ~2e-3 so
                # bf16 rounding is ~1e-5 absolute on the output)
                nc.scalar.activation(w2row[:, cj * 512:(cj + 1) * 512],
                                     w2ps[:], Copy, bias=0.0, scale=w2_scale)
            w2rep = wpool.tile([P, C], f32)
            for cj in range(C // 512):
                w2rp = psmall.tile([P, 512], f32, tag="w2ps", bufs=1,
                                   name=f"w2rp{cj}")
                nc.tensor.matmul(w2rp[:], ones1_b[:],
                                 w2row[:, cj * 512:(cj + 1) * 512],
                                 start=True, stop=True)
                nc.scalar.activation(w2rep[:, cj * 512:(cj + 1) * 512],
                                     w2rp[:], Copy, bias=0.0, scale=1.0)

            # ---- pipelined chunks: prep runs 2 chunks ahead of compute
            # so chunk-boundary cast/square/tree latency never stalls PE ----
            chunk_state = {0: (xbf0, t1s0)}
            chunk_state[1] = prep(1)
            for ch in range(BSH // BCH):
                xbf, t1s = chunk_state.pop(ch)
                compute(ch, xbf, t1s)
                if ch + 2 < BSH // BCH:
                    chunk_state[ch + 2] = prep(ch + 2)

    nc.compile()
    return nc


def _get_nc():
    if "nc" not in _CACHE:
        _CACHE["nc"] = _build_nc()
    return _CACHE["nc"]


def _prep_inputs(x, W):
    x = np.ascontiguousarray(x, dtype=np.float32)
    W = np.ascontiguousarray(W, dtype=np.float32)
    if MM_MODE == "fp8":
        # prescale by 16 to keep W out of the e4m3 subnormal range; the
        # kernel's epilogue scale folds the 1/16 back out
        wT = np.ascontiguousarray(W.T * np.float32(16.0)).astype(
            ml_dtypes.float8_e4m3
        )
    else:
        wT = np.ascontiguousarray(W.T).astype(ml_dtypes.bfloat16)
    in_maps = []
    for i in range(NCORES):
        xT_i = np.ascontiguousarray(x[i * BSH:(i + 1) * BSH, :].T)
        in_maps.append({"xT": xT_i, "wT": wT})
    return in_maps


def run(x, W, trace=False, **trace_kwargs):
    """Run on the 8 cores; returns (out [B, C] f32, BassKernelResults)."""
    from concourse import bass_utils

    nc = _get_nc()
    in_maps = _prep_inputs(x, W)
    res = bass_utils.run_bass_kernel_spmd(
        nc, in_maps, core_ids=list(range(NCORES)), trace=trace, **trace_kwargs
    )
    out = np.concatenate([r["y"] for r in res.results], axis=0)
    return out, res


def kernel(x, W, task_id=None, **_unused):
    out, _ = run(np.asarray(x), np.asarray(W), trace=False)
    return out



# revision 3
# speedup vs baseline: 1.1023x; 1.1023x over previous
"""EuclideanDeconf kernel for 8x TRN2 NeuronCores (v2).

Computes out[b, c] = (2/D) * x @ W.T - ||x||^2/D - ||W||^2/D
for x [16384, 1024] f32, W [2048, 1024] f32 -> out [16384, 2048] f32.

Sharding: data-parallel over batch. Each of the 8 cores gets 2048 rows of
x and the full W. Host does layout-only work (transpose / cast / shard /
concat); all FLOPs (matmul, norms, combine) run on device.

v2 structure (vs the 124us v1): HBM traffic cut 26MB -> 18MB/core (x and W
shipped pre-cast fp8 for the matmul + bf16 row-major copies for the norm
reductions; y stored bf16 and upcast on host), all x2/w2 work moved off
the PE/ACT hot paths onto single-pass DVE tensor_tensor_reduce ops, and
the two HWDGE rings split (inputs on SP, wRd + y-out on ACT ring).

Per-core engine plan:
  PE:    12 warmup bf16 matmuls (HAM un-throttle) + 256 e4m3 DoubleRow
         matmuls (the 8.6 GFLOP core; ~220ns each when fed) - nothing else
  DVE:   16 x2 reduces + 16 w2 reduces (tensor_tensor_reduce: square,
         scale, row-sum in one op) + 16 pass2 adds (y = t + (-w2rep), all
         bf16, one [128,2048] op per b-tile)
  ACT:   32 pass1 ops (t = cross_scale*psum - x2[b] via Identity bias) +
         y-out/wRd DMA issues on the ACT HWDGE ring
  GPSIMD: partition_broadcast of the -w2 row to [128, C] (SWDGE)
  DMA:   ring1 (SP): x fp8 chunks, W fp8 cj-blocks, xRd bf16, w2row gather
         ring2 (ACT): wRd bf16, y-out bf16

Host layouts are p-major so every DMA lands with >=2KB/partition
descriptors. Numerics: cross term via e4m3 (x plain, W prescaled by 16;
epilogue scale folds it out), x2/w2 from bf16 copies reduced in f32, y
stored bf16 -> norm rel err ~1e-3 (gate 2e-2).
"""

import numpy as np
import ml_dtypes

# Problem constants (hardcoded; kernel.py must be self-contained).
B, D, C = 16384, 1024, 2048
NCORES = 8
BSH = B // NCORES   # 2048 rows of x per core
P = 128             # partitions
KT = D // P         # 8 contraction k-planes
NB = BSH // P       # 16 b-tiles per core
NCH = 4             # x chunks (512 b-cols each)
BCH = BSH // NCH    # 512
NCJ = 4             # W column blocks (512 classes each)
CJW = C // NCJ      # 512

_CACHE = {}


def _build_nc():
    import concourse.tile as tile
    import concourse.mybir as mybir
    import concourse.bass as bass
    from concourse import bacc

    f32 = mybir.dt.float32
    bf16 = mybir.dt.bfloat16
    fp8 = mybir.dt.float8e4
    PSUM = bass.MemorySpace.PSUM
    Identity = mybir.ActivationFunctionType.Identity
    MULT = mybir.AluOpType.mult
    ADD = mybir.AluOpType.add
    DR = mybir.MatmulPerfMode.DoubleRow

    # W is host-prescaled by 16 (keeps e4m3 out of subnormals); the
    # epilogue scale folds the 1/16 back out.
    cross_scale = 2.0 / D / 16.0

    nc = bacc.Bacc(
        "TRN2",
        target_bir_lowering=False,
        debug=False,
        enable_asserts=False,
    )
    # p-major host layouts (see _prep_inputs)
    xTb = nc.dram_tensor("xTb", [NCH, P, KT * BCH], fp8, kind="ExternalInput").ap()
    wTb = nc.dram_tensor("wTb", [NCJ, P, KT * CJW], fp8, kind="ExternalInput").ap()
    xRd = nc.dram_tensor("xRd", [P, NB * D], bf16, kind="ExternalInput").ap()
    wRd = nc.dram_tensor("wRd", [P, (C // P) * D], bf16, kind="ExternalInput").ap()
    y = nc.dram_tensor("y", [BSH, C], bf16, kind="ExternalOutput").ap()

    with tile.TileContext(nc) as tc:
        with (
            tc.tile_pool(name="consts", bufs=1) as cpool,
            tc.tile_pool(name="wpool", bufs=1) as wpool,
            tc.tile_pool(name="xpool", bufs=1) as xpool,
            tc.tile_pool(name="rpool", bufs=1) as rpool,
            tc.tile_pool(name="spool", bufs=2) as spool,
            tc.tile_pool(name="tpool", bufs=6) as tpool,
            tc.tile_pool(name="ypool", bufs=3) as ypool,
            tc.tile_pool(name="pmain", bufs=3, space=PSUM) as pmain,
            tc.tile_pool(name="pwarm", bufs=1, space=PSUM) as pwarm,
        ):
            # ---- consts + PE warmup (covers DMA ramp, releases HAM) ----
            warmz = cpool.tile([P, 512], bf16)
            nc.gpsimd.memset(warmz[:], 0.0)
            warm_ps = pwarm.tile([P, 512], f32, tag="wps", bufs=1)
            for _ in range(12):
                nc.tensor.matmul(warm_ps[:], warmz[:, 0:P], warmz[:],
                                 start=True, stop=True)
            # touch ACT early so its function-table DMA is off the
            # critical path by the time the first pass1 runs
            warm1 = cpool.tile([1, 1], f32)
            nc.scalar.activation(warm1[:], warmz[0:1, 0:1], Identity,
                                 bias=0.0, scale=1.0)

            # ---- SBUF tiles ----
            wcj = [wpool.tile([P, KT, CJW], fp8, name=f"wcj{c}")
                   for c in range(NCJ)]
            xch = [xpool.tile([P, KT, BCH], fp8, name=f"xch{c}")
                   for c in range(NCH)]
            xrd = rpool.tile([P, NB, D], bf16)
            wrd = rpool.tile([P, C // P, D], bf16)
            x2cols = rpool.tile([P, NB], f32)
            w2cf = rpool.tile([P, C // P], f32)
            w2cb = rpool.tile([P, C // P], bf16)
            w2row = rpool.tile([1, C], bf16)
            w2rep = rpool.tile([P, C], bf16)

            # ---- ring1 (SP) input DMAs, ordered for arrival-vs-need ----
            def dma_xrd(piece):  # 2 b-tiles (0.5MB) per piece
                src = xRd[:, piece * 2 * D:(piece + 1) * 2 * D]
                nc.sync.dma_start(
                    xrd[:, 2 * piece:2 * piece + 2, :],
                    src.rearrange("p (j d) -> p j d", j=2),
                )

            nc.sync.dma_start(
                xch[0][:], xTb[0].rearrange("p (k b) -> p k b", k=KT))
            dma_xrd(0)
            for cj in range(NCJ):
                nc.sync.dma_start(
                    wcj[cj][:], wTb[cj].rearrange("p (k b) -> p k b", k=KT))
            dma_xrd(1)
            nc.sync.dma_start(
                xch[1][:], xTb[1].rearrange("p (k b) -> p k b", k=KT))
            dma_xrd(2)
            dma_xrd(3)
            nc.sync.dma_start(
                xch[2][:], xTb[2].rearrange("p (k b) -> p k b", k=KT))
            for piece in range(4, 8):
                dma_xrd(piece)
            nc.sync.dma_start(
                xch[3][:], xTb[3].rearrange("p (k b) -> p k b", k=KT))

            # ---- ring2 (ACT) : wRd early, y-out later ----
            for piece in range(2):
                src = wRd[:, piece * 8 * D:(piece + 1) * 8 * D]
                nc.scalar.dma_start(
                    wrd[:, 8 * piece:8 * piece + 8, :],
                    src.rearrange("p (j d) -> p j d", j=8),
                )

            # ---- DVE reduction helpers ----
            # accum_out = sum((in*(-1/D))*in) = -||row||^2/D in one DVE op
            def x2_reduce(j):
                scr = spool.tile([P, D], bf16, tag="scr", name=f"sx{j}")
                nc.vector.scalar_tensor_tensor(
                    out=scr[:], in0=xrd[:, j, :], scalar=-1.0 / D,
                    in1=xrd[:, j, :], op0=MULT, op1=MULT,
                    accum_out=x2cols[:, j:j + 1],
                )

            def w2_reduce(t):
                scr = spool.tile([P, D], bf16, tag="scr", name=f"sw{t}")
                nc.vector.scalar_tensor_tensor(
                    out=scr[:], in0=wrd[:, t, :], scalar=-1.0 / D,
                    in1=wrd[:, t, :], op0=MULT, op1=MULT,
                    accum_out=w2cf[:, t:t + 1],
                )

            # x2 for the first 4 b-tiles first (pass1 needs them early),
            # then the full w2 chain, then the rest of x2 interleaves with
            # pass2 in the main loop.
            for j in range(4):
                x2_reduce(j)
            for t in range(C // P):
                w2_reduce(t)
            nc.vector.tensor_copy(w2cb[:], w2cf[:])
            # gather [128, 16] partition-major -> one [1, C] row
            # (w2cb[p, t] holds -w2 of class p*16+t; row index p*16+t)
            nc.sync.dma_start(w2row[:], w2cb[:])
            # broadcast partition 0 row to all 128 partitions (SWDGE)
            nc.gpsimd.partition_broadcast(w2rep[:], w2row[:])

            # ---- main loop: 16 b-tiles of 16 DR matmuls + epilogue ----
            y2 = None
            for j in range(NB):
                ch, jl = divmod(j, NB // NCH)
                if j % 2 == 0:
                    y2 = ypool.tile([P, 2, C], bf16, tag="y2", name=f"y2_{j}")
                t_t = tpool.tile([P, C], bf16, tag="t", name=f"t{j}")
                for h in range(2):
                    ps = pmain.tile([P, 1024], f32, tag="ps", name=f"ps{j}_{h}")
                    for cj in (2 * h, 2 * h + 1):
                        for k2 in range(KT // 2):
                            nc.tensor.matmul(
                                ps[:, (cj % 2) * 512:(cj % 2) * 512 + 512],
                                xch[ch][:, 2 * k2:2 * k2 + 2,
                                        jl * P:(jl + 1) * P],
                                wcj[cj][:, 2 * k2:2 * k2 + 2, :],
                                start=(k2 == 0),
                                stop=(k2 == KT // 2 - 1),
                                perf_mode=DR,
                            )
                    # pass1: t = cross_scale*psum - x2[b]
                    nc.scalar.activation(
                        t_t[:, h * 1024:(h + 1) * 1024], ps[:], Identity,
                        bias=x2cols[:, j:j + 1], scale=cross_scale)
                # interleave remaining x2 reduces ahead of their use
                if 4 + 2 * j < NB:
                    x2_reduce(4 + 2 * j)
                    if 5 + 2 * j < NB:
                        x2_reduce(5 + 2 * j)
                if j == NB - 1:
                    # last b-tile: split pass2 + store per half to shorten
                    # the tail
                    for h in range(2):
                        sl = slice(h * 1024, (h + 1) * 1024)
                        nc.vector.tensor_tensor(
                            y2[:, 1, sl], t_t[:, sl], w2rep[:, sl], op=ADD)
                        nc.scalar.dma_start(
                            y[j * P:(j + 1) * P, sl], y2[:, 1, sl])
                else:
                    # pass2: y = t + (-w2) over the whole b-tile
                    nc.vector.tensor_tensor(
                        y2[:, j % 2, :], t_t[:], w2rep[:], op=ADD)
                    if j % 2 == 1 and j < NB - 2:
                        dst = y[(j - 1) * P:(j + 1) * P, :]
                        nc.scalar.dma_start(
                            dst.rearrange("(t p) c -> p t c", p=P), y2[:])
                    elif j == NB - 2:
                        nc.scalar.dma_start(y[j * P:(j + 1) * P, :],
                                            y2[:, 0, :])

    nc.compile()
    return nc


def _get_nc():
    if "nc" not in _CACHE:
        _CACHE["nc"] = _build_nc()
    return _CACHE["nc"]


def _prep_inputs(x, W):
    x = np.ascontiguousarray(x, dtype=np.float32)
    W = np.ascontiguousarray(W, dtype=np.float32)
    e4 = ml_dtypes.float8_e4m3
    bf = ml_dtypes.bfloat16

    # W fp8 cj-blocks, p-major: wTb[cj, p, k*512 + b] = 16*W.T[k*128+p,
    # cj*512+b]
    wT16 = (W.T * np.float32(16.0)).astype(e4)          # [D, C]
    wTb = np.ascontiguousarray(
        wT16.reshape(KT, P, NCJ, CJW).transpose(2, 1, 0, 3)
    ).reshape(NCJ, P, KT * CJW)
    # W bf16 rows, p-major by (p, t): partition p holds classes p*16+t
    wRd = np.ascontiguousarray(
        W.astype(bf).reshape(P, C // P, D)
    ).reshape(P, (C // P) * D)

    in_maps = []
    for i in range(NCORES):
        xs = x[i * BSH:(i + 1) * BSH, :]                # [2048, 1024]
        xT8 = xs.T.astype(e4)                           # [D, BSH]
        xTbi = np.ascontiguousarray(
            xT8.reshape(KT, P, NCH, BCH).transpose(2, 1, 0, 3)
        ).reshape(NCH, P, KT * BCH)
        xRdi = np.ascontiguousarray(
            xs.astype(bf).reshape(NB, P, D).transpose(1, 0, 2)
        ).reshape(P, NB * D)
        in_maps.append({"xTb": xTbi, "xRd": xRdi, "wTb": wTb, "wRd": wRd})
    return in_maps


def run(x, W, trace=False, **trace_kwargs):
    """Run on the 8 cores; returns (out [B, C] f32, BassKernelResults)."""
    from concourse import bass_utils

    nc = _get_nc()
    in_maps = _prep_inputs(x, W)
    res = bass_utils.run_bass_kernel_spmd(
        nc, in_maps, core_ids=list(range(NCORES)), trace=trace, **trace_kwargs
    )
    out = np.concatenate(
        [r["y"].astype(np.float32) for r in res.results], axis=0)
    return out, res


def kernel(x, W, task_id=None, **_unused):
    out, _ = run(np.asarray(x), np.asarray(W), trace=False)
    return out
